# revision 5
# baseline (speedup 1.0000x reference)
"""AffCoeffToMatrix TRN2 kernel (v2: quarter-angle + 3-engine balance).

For each batch element (B = 2,000,000):
  R = rodrigues(rotat), U = rodrigues(scal_dir), D = exp(scal)
  M = R @ (U @ diag(D) @ U^T);  out = [M | trans]  -> [B, 3, 4] f32

Math per rotation (quaternion form, constants folded into ACT scale/bias):
  v6 = 2*v (deinterleave, ACT scale)        th2 = |v6|^2 = 4 theta^2
  lg = ln(th2); th = exp(.5 lg - ln2) = theta; rth2 = exp(-.5 lg + ln sqrt2)
  s4 = sin(th/4), c4 = sin(th/4 + pi/2)     [no range wrap needed]
  sh2 = 2 s4 c4 = sin(th/2); w2 = sqrt2 cos(th/2) = -2sqrt2 s4^2 + sqrt2
  costh = 1 - 2 sh2^2;  t2 = sh2*rth2;  Q = t2*v6 = sqrt2 sin(th/2)/th * v
  R = costh*I + Q Q^T + [w2*Q]_x
Scaling: e = exp(s/2), W = U diag(e), S = W W^T (6 unique), M = R S.

Sharding: pure batch-parallel over 8 NeuronCores. On-core: [128 part x F free]
planes, fp16 matrix phase, paired (R|U) scalar chain, work split across
DVE / ACT / Pool(gpsimd) engines.
"""
import math
import sys

for _p in ("/opt/trn_rl_repo", "/root/.axon_site/_ro/trn_rl_repo"):
    if _p not in sys.path:
        sys.path.append(_p)

import numpy as np

import concourse.bass as bass
import concourse.mybir as mybir
import concourse.tile as tile

F32 = mybir.dt.float32
F16 = mybir.dt.float16
AF = mybir.ActivationFunctionType
OP = mybir.AluOpType
PI = math.pi
LN2 = math.log(2.0)
LNS2 = math.log(math.sqrt(2.0))
S2 = math.sqrt(2.0)

# ---- hardcoded problem geometry ----
B = 2_000_000
N_CORES = 8
P = 128
F = 328            # free-dim elements per tile
T = 6              # tiles per core
L = F * T          # elements per partition lane
E = P * L          # elements per core
BPAD = N_CORES * E

MAT_DT = F16

# engine assignment for tunable op groups: "dve" | "act" | "pool"
ASSIGN = {
    "sq6": "act",     # squares of v6 (6F)
    "sqq": "act",     # squares of Q (6F)
    "sqw": "act",     # squares of W (9F)
    "sdt": "pool",    # S diag partial add (3F)
    "sdiag": "dve",   # S diag assembly (3F)
    "q3": "pool",     # S offdiag partial add (3F)
    "soff": "dve",    # S offdiag assembly (3F)
    "out_m": "pool",  # output interleave copy of m9 (9F)
    "out_t": "pool",  # output interleave copy of trans (3F)
    "th2": "dve",     # theta^2 adds (4F)
    "msadd": "dve",   # matmul partial adds (2x9F)
}


def _split_multi_waits(nc, limit=1, drain_limit=0):
    """This container's walrus cannot encode >1 sync-wait per instruction
    (Drain: none at all). Spill extras onto same-engine NOPs."""
    for b in nc.main_func.blocks:
        new = []
        for ins in b.instructions:
            si = getattr(ins, "sync_info", None)
            waits = list(si.on_wait) if (si is not None and si.on_wait) else []
            lim = drain_limit if isinstance(ins, mybir.InstDrain) else limit
            if len(waits) > lim:
                keep, spill = waits[:lim], waits[lim:]
                for w in spill:
                    nop = mybir.InstNoOp(
                        name=nc.get_next_instruction_name(),
                        sync_info=mybir.SyncInfo(on_wait=[w], on_update=[]),
                        bass_nofuse=True,
                        engine=ins.engine,
                    )
                    nc.register_instruction(nop)
                    new.append(nop)
                ins.sync_info = mybir.SyncInfo(
                    on_wait=keep, on_update=list(si.on_update or [])
                )
            new.append(ins)
        b.instructions[:] = new
    return nc


def build_module(F=F, T=T, mat_dt=MAT_DT, loop_rep=None, assign=None):
    asn = dict(ASSIGN)
    if assign:
        asn.update(assign)
    nc = bass.Bass()
    E_ = P * F * T
    rot = nc.dram_tensor("rotat", [E_, 3], F32, kind="ExternalInput")
    sd = nc.dram_tensor("scal_dir", [E_, 3], F32, kind="ExternalInput")
    sc = nc.dram_tensor("scal", [E_, 3], F32, kind="ExternalInput")
    tr = nc.dram_tensor("trans", [E_, 3], F32, kind="ExternalInput")
    out = nc.dram_tensor("out", [E_, 12], F32, kind="ExternalOutput")

    rotv = rot[:].rearrange("(t p f) c -> t p (f c)", t=T, p=P)
    sdv = sd[:].rearrange("(t p f) c -> t p (f c)", t=T, p=P)
    scv = sc[:].rearrange("(t p f) c -> t p (f c)", t=T, p=P)
    trv = tr[:].rearrange("(t p f) c -> t p (f c)", t=T, p=P)
    outv = out[:].rearrange("(t p f) c -> t p (f c)", t=T, p=P)

    def engof(key):
        return {"dve": nc.vector, "act": nc.scalar, "pool": nc.gpsimd}[asn[key]]

    def square(key, out_ap, in_ap):
        if asn[key] == "act":
            nc.scalar.activation(out_ap, in_ap, AF.Square)
        else:
            engof(key).tensor_mul(out_ap, in_ap, in_ap)

    with tile.TileContext(nc) as tc:
        with (
            tc.tile_pool(name="pin", bufs=2) as pin,
            tc.tile_pool(name="pout", bufs=2) as pout,
            tc.tile_pool(name="pch", bufs=1) as pch,
            tc.tile_pool(name="pch2", bufs=2) as pch2,
            tc.tile_pool(name="pm2", bufs=2) as pm2,
            tc.tile_pool(name="pmat", bufs=1) as pmat,
            tc.tile_pool(name="pc", bufs=1) as pc,
        ):
            pi2 = pc.tile([P, 1], F32, tag="pi2")
            nc.vector.memset(pi2[:], PI / 2)
            nln2 = pc.tile([P, 1], F32, tag="nln2")
            nc.vector.memset(nln2[:], -LN2)
            lns2 = pc.tile([P, 1], F32, tag="lns2")
            nc.vector.memset(lns2[:], LNS2)

            F2 = 2 * F

            def part_pre(ti):
                """DMA in + deinterleave + th2 + exp-set ACT chain."""
                st = {}
                rs6 = pin.tile([P, 6 * F], F32, tag="rs6", name="rs6")
                nc.sync.dma_start(out=rs6[:, : 3 * F], in_=rotv[ti])
                nc.sync.dma_start(out=rs6[:, 3 * F :], in_=sdv[ti])
                c3 = pin.tile([P, 3 * F], F32, tag="sc3", name="sc3")
                nc.sync.dma_start(out=c3[:], in_=scv[ti])
                t3 = pin.tile([P, 3 * F], F32, tag="tr3", name="tr3")
                nc.sync.dma_start(out=t3[:], in_=trv[ti])
                st["t3"] = t3

                # deinterleave (g f c) -> (g c f), scale by 2:  v6 = 2*v
                v6 = pm2.tile([P, 6 * F], mat_dt, tag="v6", name="v6")
                v6v = v6[:].rearrange("p (g c f) -> p g c f", g=2, c=3)
                rs6v = rs6[:].rearrange("p (g f c) -> p g c f", g=2, c=3)
                nc.scalar.activation(v6v, rs6v, AF.Copy, scale=2.0)
                st["v6"] = v6

                # th2 = |v6|^2  (= 4 theta^2)
                sq6 = pch.tile([P, 6 * F], mat_dt, tag="sq6", name="sq6")
                sq6v = sq6[:].rearrange("p (g c f) -> p g c f", g=2, c=3)
                square("sq6", sq6v, v6v)
                th2a = pch.tile([P, F2], mat_dt, tag="th2a", name="th2a")
                th2av = th2a[:].rearrange("p (g f) -> p g f", g=2)
                eng_th2 = engof("th2")
                eng_th2.tensor_add(th2av, sq6v[:, :, 0, :], sq6v[:, :, 1, :])
                th2 = pch.tile([P, F2], mat_dt, tag="th2", name="th2")
                th2v = th2[:].rearrange("p (g f) -> p g f", g=2)
                eng_th2.tensor_add(th2v, th2av, sq6v[:, :, 2, :])

                # ---- natural_log_exp table set ----
                lg = pch.tile([P, F2], F32, tag="lg", name="lg")
                nc.scalar.activation(lg[:], th2[:], AF.Ln)
                th = pch2.tile([P, F2], mat_dt, tag="th", name="th")
                nc.scalar.activation(th[:], lg[:], AF.Exp, scale=0.5, bias=nln2[:])
                rth2 = pch2.tile([P, F2], mat_dt, tag="rth2", name="rth2")
                nc.scalar.activation(rth2[:], lg[:], AF.Exp, scale=-0.5, bias=lns2[:])
                e3 = pm2.tile([P, 3 * F], mat_dt, tag="e3", name="e3")
                e3v = e3[:].rearrange("p (c f) -> p c f", c=3)
                nc.scalar.activation(
                    e3v, c3[:].rearrange("p (f c) -> p c f", c=3), AF.Exp, scale=0.5
                )
                st["th"], st["rth2"], st["e3"] = th, rth2, e3
                return st

            def part_trig(st):
                """trig table set: s4 = sin(th/4), c4 = cos(th/4)."""
                th = st["th"]
                s4 = pch2.tile([P, F2], mat_dt, tag="s4", name="s4")
                nc.scalar.activation(s4[:], th[:], AF.Sin, scale=0.25)
                c4 = pch2.tile([P, F2], mat_dt, tag="c4", name="c4")
                nc.scalar.activation(c4[:], th[:], AF.Sin, scale=0.25, bias=pi2[:])
                st["s4"], st["c4"] = s4, c4

            def part_mat(ti, st):
                v6, t3 = st["v6"], st["t3"]
                s4, c4, rth2, e3 = st["s4"], st["c4"], st["rth2"], st["e3"]
                v6v = v6[:].rearrange("p (g c f) -> p g c f", g=2, c=3)
                e3v = e3[:].rearrange("p (c f) -> p c f", c=3)

                def cht(tag, w=F2):
                    return pch.tile([P, w], F32, tag=tag, name=tag)

                def mt(tag, w):
                    return pmat.tile([P, w], mat_dt, tag=tag, name=tag)

                # ---- derived scalars (fp16, narrow) ----
                sh2 = mt("sh2", F2)
                nc.vector.scalar_tensor_tensor(
                    sh2[:], s4[:], 2.0, c4[:], OP.mult, OP.mult
                )
                s4q = mt("s4q", F2)
                nc.vector.tensor_mul(s4q[:], s4[:], s4[:])
                w2 = mt("w2", F2)
                nc.vector.tensor_scalar(w2[:], s4q[:], -2.0 * S2, S2, OP.mult, OP.add)
                shq = mt("shq", F2)
                nc.vector.tensor_mul(shq[:], sh2[:], sh2[:])
                costh = mt("costh", F2)
                nc.vector.tensor_scalar(costh[:], shq[:], -2.0, 1.0, OP.mult, OP.add)
                t2 = mt("t2", F2)
                nc.vector.tensor_mul(t2[:], sh2[:], rth2[:])
                t2v = t2[:].rearrange("p (g f) -> p g f", g=2)
                w2v = w2[:].rearrange("p (g f) -> p g f", g=2)
                cthv = costh[:].rearrange("p (g f) -> p g f", g=2)

                # ---- Q = t2 * v6 ----
                Q6 = mt("Q6", 6 * F)
                Q6v = Q6[:].rearrange("p (g c f) -> p g c f", g=2, c=3)
                nc.vector.tensor_mul(
                    Q6v, t2v.unsqueeze(2).to_broadcast((P, 2, 3, F)), v6v
                )
                sqQ = mt("sqQ", 6 * F)
                sqQv = sqQ[:].rearrange("p (g c f) -> p g c f", g=2, c=3)
                square("sqq", sqQv, Q6v)

                # ---- RU18 = (R9 | U9) row-major ----
                RU18 = mt("RU18", 18 * F)
                ruv = RU18[:].rearrange("p (g k f) -> p g k f", g=2, k=9)
                # diag @ (0,4,8) = sqQ + costh
                nc.vector.tensor_add(
                    ruv[:, :, 0:9:4, :],
                    sqQv,
                    cthv.unsqueeze(2).to_broadcast((P, 2, 3, F)),
                )
                # p pairs (01, 12, 20); A aligned (z, x, y)
                p6 = mt("p6", 6 * F)
                p6v = p6[:].rearrange("p (g j f) -> p g j f", g=2, j=3)
                nc.vector.tensor_mul(
                    p6v[:, :, 0:2, :], Q6v[:, :, 0:2, :], Q6v[:, :, 1:3, :]
                )
                nc.vector.tensor_mul(p6v[:, :, 2, :], Q6v[:, :, 2, :], Q6v[:, :, 0, :])
                a6 = mt("a6", 6 * F)
                a6v = a6[:].rearrange("p (g j f) -> p g j f", g=2, j=3)
                w2b = w2v.unsqueeze(2).to_broadcast((P, 2, 2, F))
                nc.vector.tensor_mul(a6v[:, :, 0, :], w2v, Q6v[:, :, 2, :])
                nc.vector.tensor_mul(a6v[:, :, 1:3, :], w2b, Q6v[:, :, 0:2, :])
                # plus: @3 = p01+az, @7 = p12+ax; @2 = p20+ay
                nc.vector.tensor_add(
                    ruv[:, :, 3:8:4, :], p6v[:, :, 0:2, :], a6v[:, :, 0:2, :]
                )
                nc.vector.tensor_add(ruv[:, :, 2, :], p6v[:, :, 2, :], a6v[:, :, 2, :])
                # minus: @1 = p01-az, @5 = p12-ax; @6 = p20-ay
                nc.vector.tensor_sub(
                    ruv[:, :, 1:6:4, :], p6v[:, :, 0:2, :], a6v[:, :, 0:2, :]
                )
                nc.vector.tensor_sub(ruv[:, :, 6, :], p6v[:, :, 2, :], a6v[:, :, 2, :])
                R9v = RU18[:, : 9 * F].rearrange("p (k f) -> p k f", k=9)
                U9v = RU18[:, 9 * F :].rearrange("p (i k f) -> p i k f", i=3, k=3)

                # ---- W = U * diag(e);  S = W W^T ----
                W9 = mt("W9", 9 * F)
                W9v4 = W9[:].rearrange("p (i k f) -> p i k f", i=3, k=3)
                e_b = e3v.unsqueeze(1).to_broadcast((P, 3, 3, F))
                nc.vector.tensor_mul(W9v4, U9v, e_b)
                W9v = W9[:].rearrange("p (k f) -> p k f", k=9)
                sqW = mt("sqW", 9 * F)
                square("sqw", sqW[:], W9[:])
                sqWv = sqW[:].rearrange("p (i k f) -> p i k f", i=3, k=3)
                # S unique-6 layout with holes: S00@0 S01@1 S02@2 S11@3 S12@5 S22@8
                S9 = mt("S9", 9 * F)
                S9v = S9[:].rearrange("p (k f) -> p k f", k=9)
                sdt = mt("sdt", 3 * F)
                sdtv = sdt[:].rearrange("p (c f) -> p c f", c=3)
                engof("sdt").tensor_add(sdtv, sqWv[:, :, 0, :], sqWv[:, :, 1, :])
                eng_sdiag = engof("sdiag")
                eng_sdiag.tensor_add(
                    S9v[:, 0:4:3, :], sdtv[:, 0:2, :], sqWv[:, 0:2, 2, :]
                )
                eng_sdiag.tensor_add(S9v[:, 8, :], sdtv[:, 2, :], sqWv[:, 2, 2, :])
                # off-diagonal: pp[g] = Wrow_i * Wrow_j for (01, 02, 12)
                pp = mt("pp", 9 * F)
                ppv = pp[:].rearrange("p (g k f) -> p g k f", g=3, k=3)
                w0b = (
                    W9v[:, 0:3, :].unsqueeze(1).to_broadcast((P, 2, 3, F))
                )
                nc.vector.tensor_mul(
                    ppv[:, 0:2, :, :], w0b, W9v4[:, 1:3, :, :]
                )
                nc.vector.tensor_mul(ppv[:, 2, :, :], W9v[:, 3:6, :], W9v[:, 6:9, :])
                q3 = mt("q3", 3 * F)
                q3v = q3[:].rearrange("p (g f) -> p g f", g=3)
                engof("q3").tensor_add(q3v, ppv[:, :, 0, :], ppv[:, :, 1, :])
                eng_soff = engof("soff")
                eng_soff.tensor_add(
                    S9v[:, 1:3, :], q3v[:, 0:2, :], ppv[:, 0:2, 2, :]
                )
                eng_soff.tensor_add(S9v[:, 5, :], q3v[:, 2, :], ppv[:, 2, 2, :])

                # ---- M = R @ S  (5 wide instructions) ----
                srow = [
                    S9v[:, 0:3, :].unsqueeze(1).to_broadcast((P, 3, 3, F)),
                    S9v[:, 1:7:2, :].unsqueeze(1).to_broadcast((P, 3, 3, F)),
                    S9v[:, 2:9:3, :].unsqueeze(1).to_broadcast((P, 3, 3, F)),
                ]

                def rcol(k):
                    return (
                        R9v[:, k:9:3, :]
                        .unsqueeze(2)
                        .to_broadcast((P, 3, 3, F))
                    )

                mp1 = mt("mp1", 9 * F)
                mp1v = mp1[:].rearrange("p (r k f) -> p r k f", r=3, k=3)
                nc.vector.tensor_mul(mp1v, rcol(0), srow[0])
                mp2 = mt("mp2", 9 * F)
                mp2v = mp2[:].rearrange("p (r k f) -> p r k f", r=3, k=3)
                nc.vector.tensor_mul(mp2v, rcol(1), srow[1])
                ms = mt("ms", 9 * F)
                msv = ms[:].rearrange("p (r k f) -> p r k f", r=3, k=3)
                engof("msadd").tensor_add(msv, mp1v, mp2v)
                mp3 = mt("mp3", 9 * F)
                mp3v = mp3[:].rearrange("p (r k f) -> p r k f", r=3, k=3)
                nc.vector.tensor_mul(mp3v, rcol(2), srow[2])
                m9 = pmat.tile([P, 9 * F], mat_dt, tag="m9", name="m9")
                m9v = m9[:].rearrange("p (r k f) -> p r k f", r=3, k=3)
                engof("msadd").tensor_add(m9v, msv, mp3v)

                # ---- interleave to f32 out: rows + trans ----
                ot = pout.tile([P, 12 * F], F32, tag="out", name="ot")
                # out element (f, c): offset 12f + c;  c = 4r + k (k<3) or 4r+3
                otm = ot[:].rearrange("p (f r k) -> p r k f", r=3, k=4)
                eng_om = engof("out_m")
                if asn["out_m"] == "act":
                    nc.scalar.activation(otm[:, :, 0:3, :], m9v, AF.Copy)
                else:
                    eng_om.tensor_copy(otm[:, :, 0:3, :], m9v)
                t3v = t3[:].rearrange("p (f c) -> p c f", c=3)
                eng_ot = engof("out_t")
                if asn["out_t"] == "act":
                    nc.scalar.activation(otm[:, :, 3, :], t3v, AF.Copy)
                else:
                    eng_ot.tensor_copy(otm[:, :, 3, :], t3v)
                nc.sync.dma_start(out=outv[ti], in_=ot[:])

            def body():
                assert T % 2 == 0
                for g in range(0, T, 2):
                    stA = part_pre(g)
                    stB = part_pre(g + 1)
                    part_trig(stA)
                    part_trig(stB)
                    part_mat(g, stA)
                    part_mat(g + 1, stB)

            if loop_rep is None:
                body()
            else:
                with tc.For_i(0, loop_rep, 1, staggered_reset=True):
                    body()

    _split_multi_waits(nc)
    return nc


# ----------------------------------------------------------------------------
# host-side execution
# ----------------------------------------------------------------------------
_CACHE = {}


def _get_runner():
    if "runner" in _CACHE:
        return _CACHE["runner"]
    import jax
    from jax.sharding import Mesh, PartitionSpec
    from jax.experimental.shard_map import shard_map
    from concourse.bass2jax import (
        _bass_exec_p,
        install_neuronx_cc_hook,
        partition_id_tensor,
    )

    nc = build_module()
    install_neuronx_cc_hook()
    partition_name = nc.partition_id_tensor.name if nc.partition_id_tensor else None
    in_names, out_names, out_avals, zero_outs = [], [], [], []
    for alloc in nc.m.functions[0].allocations:
        if not isinstance(alloc, mybir.MemoryLocationSet):
            continue
        name = alloc.memorylocations[0].name
        if alloc.kind == "ExternalInput":
            if name != partition_name:
                in_names.append(name)
        elif alloc.kind == "ExternalOutput":
            shape = tuple(alloc.tensor_shape)
            dtype = mybir.dt.np(alloc.dtype)
            out_names.append(name)
            out_avals.append(jax.core.ShapedArray(shape, dtype))
            zero_outs.append(np.zeros(shape, dtype))
    n_params = len(in_names)
    all_in_names = in_names + out_names + (
        [partition_name] if partition_name else []
    )

    def _body(*args):
        operands = list(args)
        if partition_name is not None:
            operands.append(partition_id_tensor())
        outs = _bass_exec_p.bind(
            *operands,
            out_avals=tuple(out_avals),
            in_names=tuple(all_in_names),
            out_names=tuple(out_names),
            lowering_input_output_aliases=(),
            sim_require_finite=True,
            sim_require_nnan=True,
            nc=nc,
        )
        return tuple(outs)

    devices = jax.devices()[:N_CORES]
    mesh = Mesh(np.asarray(devices), ("core",))
    n_outs = len(out_names)
    jf = jax.jit(
        shard_map(
            _body,
            mesh=mesh,
            in_specs=(PartitionSpec("core"),) * (n_params + n_outs),
            out_specs=(PartitionSpec("core"),) * n_outs,
            check_rep=False,
        ),
        donate_argnums=tuple(range(n_params, n_params + n_outs)),
        keep_unused=True,
    )
    _CACHE["runner"] = (jf, in_names, out_names, zero_outs)
    return _CACHE["runner"]


def kernel(trans, rotat, scal_dir, scal):
    jf, in_names, out_names, zero_outs = _get_runner()
    inputs = {"trans": trans, "rotat": rotat, "scal_dir": scal_dir, "scal": scal}
    # pad to BPAD with ones (zeros would make |v| = 0 -> inf/NaN chains)
    padded = {}
    for k, v in inputs.items():
        a = np.ones((BPAD, 3), dtype=np.float32)
        a[:B] = v
        padded[k] = a
    args = [padded[n] for n in in_names]
    zeros = [np.zeros((N_CORES * z.shape[0], *z.shape[1:]), z.dtype) for z in zero_outs]
    outs = jf(*args, *zeros)
    full = np.asarray(outs[0])  # [BPAD, 12]
    return full[:B].reshape(B, 3, 4).astype(np.float32, copy=False)


if __name__ == "__main__":
    rng = np.random.default_rng(0)
    ins = {
        "trans": rng.normal(size=(B, 3)).astype(np.float32),
        "rotat": rng.normal(size=(B, 3)).astype(np.float32),
        "scal_dir": rng.normal(size=(B, 3)).astype(np.float32),
        "scal": rng.normal(size=(B, 3)).astype(np.float32),
    }
    out = kernel(**ins)
    print(out.shape, out.dtype)


# revision 16
# speedup vs baseline: 1.2566x; 1.2566x over previous
"""AffCoeffToMatrix TRN2 kernel (v2: quarter-angle + 3-engine balance).

For each batch element (B = 2,000,000):
  R = rodrigues(rotat), U = rodrigues(scal_dir), D = exp(scal)
  M = R @ (U @ diag(D) @ U^T);  out = [M | trans]  -> [B, 3, 4] f32

Math per rotation (quaternion form, constants folded into ACT scale/bias):
  v6 = 2*v (deinterleave, ACT scale)        th2 = |v6|^2 = 4 theta^2
  lg = ln(th2); th = exp(.5 lg - ln2) = theta; rth2 = exp(-.5 lg + ln sqrt2)
  s4 = sin(th/4), c4 = sin(th/4 + pi/2)     [no range wrap needed]
  sh2 = 2 s4 c4 = sin(th/2); w2 = sqrt2 cos(th/2) = -2sqrt2 s4^2 + sqrt2
  costh = 1 - 2 sh2^2;  t2 = sh2*rth2;  Q = t2*v6 = sqrt2 sin(th/2)/th * v
  R = costh*I + Q Q^T + [w2*Q]_x
Scaling: e = exp(s/2), W = U diag(e), S = W W^T (6 unique), M = R S.

Sharding: pure batch-parallel over 8 NeuronCores. On-core: [128 part x F free]
planes, fp16 matrix phase, paired (R|U) scalar chain, work split across
DVE / ACT / Pool(gpsimd) engines.
"""
import math
import sys

for _p in ("/opt/trn_rl_repo", "/root/.axon_site/_ro/trn_rl_repo"):
    if _p not in sys.path:
        sys.path.append(_p)

import numpy as np

import concourse.bass as bass
import concourse.mybir as mybir
import concourse.tile as tile

F32 = mybir.dt.float32
F16 = mybir.dt.float16
AF = mybir.ActivationFunctionType
OP = mybir.AluOpType
PI = math.pi
LN2 = math.log(2.0)
LNS2 = math.log(math.sqrt(2.0))
S2 = math.sqrt(2.0)

# ---- hardcoded problem geometry ----
B = 2_000_000
N_CORES = 8
P = 128
F = 328            # free-dim elements per tile
T = 6              # tiles per core
L = F * T          # elements per partition lane
E = P * L          # elements per core
BPAD = N_CORES * E

MAT_DT = F16

# engine assignment for tunable op groups: "dve" | "act" | "pool"
ASSIGN = {
    "sq6": "act",     # squares of v6 (6F)
    "sqq": "act",     # squares of Q (6F)
    "sqw": "act",     # squares of W (9F)
    "sdt": "dve",     # S diag partial add (3F)
    "sdiag": "dve",   # S diag assembly (3F)
    "q3": "dve",      # S offdiag partial add (3F)
    "soff": "dve",    # S offdiag assembly (3F)
    "out_m": "act",   # output interleave copy of m9 (9F)
    "out_t": "act",   # output interleave copy of trans (3F)
    "th2": "pool",    # theta^2 adds (4F)
    "msadd": "dve",   # matmul partial adds (2x9F)
    "shq": "act",     # sh2^2 (2F)
    "order": "mats_then_front",
}


def _split_multi_waits(nc, limit=1, drain_limit=0):
    """This container's walrus cannot encode >1 sync-wait per instruction
    (Drain: none at all). Spill extras onto same-engine NOPs."""
    for b in nc.main_func.blocks:
        new = []
        for ins in b.instructions:
            si = getattr(ins, "sync_info", None)
            waits = list(si.on_wait) if (si is not None and si.on_wait) else []
            lim = drain_limit if isinstance(ins, mybir.InstDrain) else limit
            if len(waits) > lim:
                keep, spill = waits[:lim], waits[lim:]
                for w in spill:
                    nop = mybir.InstNoOp(
                        name=nc.get_next_instruction_name(),
                        sync_info=mybir.SyncInfo(on_wait=[w], on_update=[]),
                        bass_nofuse=True,
                        engine=ins.engine,
                    )
                    nc.register_instruction(nop)
                    new.append(nop)
                ins.sync_info = mybir.SyncInfo(
                    on_wait=keep, on_update=list(si.on_update or [])
                )
            new.append(ins)
        b.instructions[:] = new
    return nc


def build_module(F=F, T=T, mat_dt=MAT_DT, loop_rep=None, assign=None):
    asn = dict(ASSIGN)
    if assign:
        asn.update(assign)
    nc = bass.Bass()
    E_ = P * F * T
    rot = nc.dram_tensor("rotat", [E_, 3], F32, kind="ExternalInput")
    sd = nc.dram_tensor("scal_dir", [E_, 3], F32, kind="ExternalInput")
    sc = nc.dram_tensor("scal", [E_, 3], F32, kind="ExternalInput")
    tr = nc.dram_tensor("trans", [E_, 3], F32, kind="ExternalInput")
    out = nc.dram_tensor("out", [E_, 12], F32, kind="ExternalOutput")

    rotv = rot[:].rearrange("(t p f) c -> t p (f c)", t=T, p=P)
    sdv = sd[:].rearrange("(t p f) c -> t p (f c)", t=T, p=P)
    scv = sc[:].rearrange("(t p f) c -> t p (f c)", t=T, p=P)
    trv = tr[:].rearrange("(t p f) c -> t p (f c)", t=T, p=P)
    outv = out[:].rearrange("(t p f) c -> t p (f c)", t=T, p=P)

    def engof(key):
        return {"dve": nc.vector, "act": nc.scalar, "pool": nc.gpsimd}[asn[key]]

    def square(key, out_ap, in_ap):
        if asn[key] == "act":
            nc.scalar.activation(out_ap, in_ap, AF.Square)
        else:
            engof(key).tensor_mul(out_ap, in_ap, in_ap)

    with tile.TileContext(nc) as tc:
        with (
            tc.tile_pool(name="pc", bufs=1) as pc,
            tc.tile_pool(name="pin", bufs=2) as pin,
            tc.tile_pool(name="pout", bufs=2) as pout,
            tc.tile_pool(name="pch", bufs=2) as pch,
            tc.tile_pool(name="pch2", bufs=4) as pch2,
            tc.tile_pool(name="pm2", bufs=4) as pm2,
            tc.tile_pool(name="pmat", bufs=1) as pmat,
        ):
            npi2 = pc.tile([P, 1], F32, tag="npi2")
            nc.vector.memset(npi2[:], -PI / 2)
            nln2 = pc.tile([P, 1], F32, tag="nln2")
            nc.vector.memset(nln2[:], -LN2)
            lns2 = pc.tile([P, 1], F32, tag="lns2")
            nc.vector.memset(lns2[:], LNS2)

            F2 = 2 * F

            def part_pre_front(ti, th2_eng=None):
                """DMA in + deinterleave + squares + th2 (Pool)."""
                st = {}
                rs6 = pin.tile([P, 6 * F], F32, tag="rs6", name="rs6")
                nc.sync.dma_start(out=rs6[:, : 3 * F], in_=rotv[ti])
                nc.sync.dma_start(out=rs6[:, 3 * F :], in_=sdv[ti])
                c3 = pin.tile([P, 3 * F], F32, tag="sc3", name="sc3")
                nc.sync.dma_start(out=c3[:], in_=scv[ti])
                st["c3"] = c3

                # deinterleave (g f c) -> (g c f), scale by 2:  v6 = 2*v
                v6 = pm2.tile([P, 6 * F], mat_dt, tag="v6", name="v6")
                v6v = v6[:].rearrange("p (g c f) -> p g c f", g=2, c=3)
                rs6v = rs6[:].rearrange("p (g f c) -> p g c f", g=2, c=3)
                nc.scalar.activation(v6v, rs6v, AF.Copy, scale=2.0)
                st["v6"] = v6

                # th2 = |v6|^2  (= 4 theta^2)
                sq6 = pch.tile([P, 6 * F], mat_dt, tag="sq6", name="sq6")
                sq6v = sq6[:].rearrange("p (g c f) -> p g c f", g=2, c=3)
                square("sq6", sq6v, v6v)
                th2a = pch.tile([P, F2], mat_dt, tag="th2a", name="th2a")
                th2av = th2a[:].rearrange("p (g f) -> p g f", g=2)
                eng_th2 = (
                    {"dve": nc.vector, "act": nc.scalar, "pool": nc.gpsimd}[th2_eng]
                    if th2_eng
                    else engof("th2")
                )
                eng_th2.tensor_add(th2av, sq6v[:, :, 0, :], sq6v[:, :, 1, :])
                th2 = pch.tile([P, F2], mat_dt, tag="th2", name="th2")
                th2v = th2[:].rearrange("p (g f) -> p g f", g=2)
                eng_th2.tensor_add(th2v, th2av, sq6v[:, :, 2, :])
                st["th2"] = th2
                return st

            def part_pre_exp(st):
                """natural_log_exp table set chain."""
                th2, c3 = st["th2"], st["c3"]
                lg = pch.tile([P, F2], mat_dt, tag="lg", name="lg")
                nc.scalar.activation(lg[:], th2[:], AF.Ln)
                th = pch2.tile([P, F2], mat_dt, tag="th", name="th")
                nc.scalar.activation(th[:], lg[:], AF.Exp, scale=0.5, bias=nln2[:])
                rth2 = pch2.tile([P, F2], mat_dt, tag="rth2", name="rth2")
                nc.scalar.activation(rth2[:], lg[:], AF.Exp, scale=-0.5, bias=lns2[:])
                e3 = pm2.tile([P, 3 * F], mat_dt, tag="e3", name="e3")
                e3v = e3[:].rearrange("p (c f) -> p c f", c=3)
                nc.scalar.activation(
                    e3v, c3[:].rearrange("p (f c) -> p c f", c=3), AF.Exp, scale=0.5
                )
                st["th"], st["rth2"], st["e3"] = th, rth2, e3

            def part_trig(st):
                """trig set: sh2 = sin(th/2), wn = -cos(th/2); shq filler."""
                th = st["th"]
                sh2 = pch2.tile([P, F2], mat_dt, tag="sh2", name="sh2")
                nc.scalar.activation(sh2[:], th[:], AF.Sin, scale=0.5)
                wn = pch2.tile([P, F2], mat_dt, tag="wn", name="wn")
                nc.scalar.activation(wn[:], th[:], AF.Sin, scale=0.5, bias=npi2[:])
                shq = pch2.tile([P, F2], mat_dt, tag="shq", name="shq")
                square("shq", shq[:], sh2[:])
                st["sh2"], st["wn"], st["shq"] = sh2, wn, shq

            def part_mat(ti, st):
                v6 = st["v6"]
                sh2, wn, shq = st["sh2"], st["wn"], st["shq"]
                rth2, e3 = st["rth2"], st["e3"]
                v6v = v6[:].rearrange("p (g c f) -> p g c f", g=2, c=3)
                e3v = e3[:].rearrange("p (c f) -> p c f", c=3)

                t3 = pin.tile([P, 3 * F], F32, tag="tr3", name="tr3")
                nc.sync.dma_start(out=t3[:], in_=trv[ti])

                def mt(tag, w):
                    return pmat.tile([P, w], mat_dt, tag=tag, name=tag)

                # ---- derived scalars ----
                costh = mt("costh", F2)
                nc.vector.tensor_scalar(costh[:], shq[:], -2.0, 1.0, OP.mult, OP.add)
                t2 = mt("t2", F2)
                nc.vector.tensor_mul(t2[:], sh2[:], rth2[:])
                t2v = t2[:].rearrange("p (g f) -> p g f", g=2)
                wnv = wn[:].rearrange("p (g f) -> p g f", g=2)
                cthv = costh[:].rearrange("p (g f) -> p g f", g=2)

                # ---- Q = t2 * v6;  sqQ on ACT right after ----
                Q6 = mt("Q6", 6 * F)
                Q6v = Q6[:].rearrange("p (g c f) -> p g c f", g=2, c=3)
                nc.vector.tensor_mul(
                    Q6v, t2v.unsqueeze(2).to_broadcast((P, 2, 3, F)), v6v
                )
                sqQ = mt("sqQ", 6 * F)
                sqQv = sqQ[:].rearrange("p (g c f) -> p g c f", g=2, c=3)
                square("sqq", sqQv, Q6v)

                RU18 = mt("RU18", 18 * F)
                ruv = RU18[:].rearrange("p (g k f) -> p g k f", g=2, k=9)
                # p pairs (01, 12, 20); a6n = -sqrt2*cos(th/2)*Q aligned (z, x, y)
                p6 = mt("p6", 6 * F)
                p6v = p6[:].rearrange("p (g j f) -> p g j f", g=2, j=3)
                nc.vector.tensor_mul(
                    p6v[:, :, 0:2, :], Q6v[:, :, 0:2, :], Q6v[:, :, 1:3, :]
                )
                nc.vector.tensor_mul(p6v[:, :, 2, :], Q6v[:, :, 2, :], Q6v[:, :, 0, :])
                a6 = mt("a6", 6 * F)
                a6v = a6[:].rearrange("p (g j f) -> p g j f", g=2, j=3)
                nc.vector.scalar_tensor_tensor(
                    a6v[:, :, 0, :], Q6v[:, :, 2, :], S2, wnv, OP.mult, OP.mult
                )
                nc.vector.scalar_tensor_tensor(
                    a6v[:, :, 1, :], Q6v[:, :, 0, :], S2, wnv, OP.mult, OP.mult
                )
                nc.vector.scalar_tensor_tensor(
                    a6v[:, :, 2, :], Q6v[:, :, 1, :], S2, wnv, OP.mult, OP.mult
                )
                # a6 holds NEGATED a-terms -> swap add/sub:
                # plus(@3,@7,@2) = p - a6n;  minus(@1,@5,@6) = p + a6n
                nc.vector.tensor_sub(
                    ruv[:, :, 3:8:4, :], p6v[:, :, 0:2, :], a6v[:, :, 0:2, :]
                )
                nc.vector.tensor_sub(ruv[:, :, 2, :], p6v[:, :, 2, :], a6v[:, :, 2, :])
                nc.vector.tensor_add(
                    ruv[:, :, 1:6:4, :], p6v[:, :, 0:2, :], a6v[:, :, 0:2, :]
                )
                nc.vector.tensor_add(ruv[:, :, 6, :], p6v[:, :, 2, :], a6v[:, :, 2, :])
                # diag @ (0,4,8) = sqQ + costh  (ACT had time for sqQ by now)
                nc.vector.tensor_add(
                    ruv[:, :, 0:9:4, :],
                    sqQv,
                    cthv.unsqueeze(2).to_broadcast((P, 2, 3, F)),
                )
                R9v = RU18[:, : 9 * F].rearrange("p (k f) -> p k f", k=9)
                U9v = RU18[:, 9 * F :].rearrange("p (i k f) -> p i k f", i=3, k=3)

                # ---- W = U * diag(e);  sqW on ACT; S = W W^T ----
                W9 = mt("W9", 9 * F)
                W9v4 = W9[:].rearrange("p (i k f) -> p i k f", i=3, k=3)
                e_b = e3v.unsqueeze(1).to_broadcast((P, 3, 3, F))
                nc.vector.tensor_mul(W9v4, U9v, e_b)
                W9v = W9[:].rearrange("p (k f) -> p k f", k=9)
                sqW = mt("sqW", 9 * F)
                square("sqw", sqW[:], W9[:])
                sqWv = sqW[:].rearrange("p (i k f) -> p i k f", i=3, k=3)
                # off-diagonal first (gives ACT time for sqW)
                pp = mt("pp", 9 * F)
                ppv = pp[:].rearrange("p (g k f) -> p g k f", g=3, k=3)
                w0b = W9v[:, 0:3, :].unsqueeze(1).to_broadcast((P, 2, 3, F))
                nc.vector.tensor_mul(ppv[:, 0:2, :, :], w0b, W9v4[:, 1:3, :, :])
                nc.vector.tensor_mul(ppv[:, 2, :, :], W9v[:, 3:6, :], W9v[:, 6:9, :])
                # S unique-6 with holes: S00@0 S01@1 S02@2 S11@3 S12@5 S22@8
                S9 = mt("S9", 9 * F)
                S9v = S9[:].rearrange("p (k f) -> p k f", k=9)
                q3 = mt("q3", 3 * F)
                q3v = q3[:].rearrange("p (g f) -> p g f", g=3)
                engof("q3").tensor_add(q3v, ppv[:, :, 0, :], ppv[:, :, 1, :])
                eng_soff = engof("soff")
                eng_soff.tensor_add(
                    S9v[:, 1:3, :], q3v[:, 0:2, :], ppv[:, 0:2, 2, :]
                )
                eng_soff.tensor_add(S9v[:, 5, :], q3v[:, 2, :], ppv[:, 2, 2, :])
                sdt = mt("sdt", 3 * F)
                sdtv = sdt[:].rearrange("p (c f) -> p c f", c=3)
                engof("sdt").tensor_add(sdtv, sqWv[:, :, 0, :], sqWv[:, :, 1, :])
                eng_sdiag = engof("sdiag")
                eng_sdiag.tensor_add(
                    S9v[:, 0:4:3, :], sdtv[:, 0:2, :], sqWv[:, 0:2, 2, :]
                )
                eng_sdiag.tensor_add(S9v[:, 8, :], sdtv[:, 2, :], sqWv[:, 2, 2, :])

                # ---- M = R @ S  (5 wide instructions) ----
                srow = [
                    S9v[:, 0:3, :].unsqueeze(1).to_broadcast((P, 3, 3, F)),
                    S9v[:, 1:7:2, :].unsqueeze(1).to_broadcast((P, 3, 3, F)),
                    S9v[:, 2:9:3, :].unsqueeze(1).to_broadcast((P, 3, 3, F)),
                ]

                def rcol(k):
                    return (
                        R9v[:, k:9:3, :]
                        .unsqueeze(2)
                        .to_broadcast((P, 3, 3, F))
                    )

                mp1 = mt("mp1", 9 * F)
                mp1v = mp1[:].rearrange("p (r k f) -> p r k f", r=3, k=3)
                nc.vector.tensor_mul(mp1v, rcol(0), srow[0])
                mp2 = mt("mp2", 9 * F)
                mp2v = mp2[:].rearrange("p (r k f) -> p r k f", r=3, k=3)
                nc.vector.tensor_mul(mp2v, rcol(1), srow[1])
                eng_ms = engof("msadd")
                eng_ms.tensor_add(mp1v, mp1v, mp2v)  # ms in-place on mp1
                mp3 = mt("mp3", 9 * F)
                mp3v = mp3[:].rearrange("p (r k f) -> p r k f", r=3, k=3)
                nc.vector.tensor_mul(mp3v, rcol(2), srow[2])
                m9 = pmat.tile([P, 9 * F], mat_dt, tag="m9", name="m9")
                m9v = m9[:].rearrange("p (r k f) -> p r k f", r=3, k=3)
                eng_ms.tensor_add(m9v, mp1v, mp3v)

                # ---- interleave to f32 out: rows + trans ----
                ot = pout.tile([P, 12 * F], F32, tag="out", name="ot")
                otm = ot[:].rearrange("p (f r k) -> p r k f", r=3, k=4)
                if asn["out_m"] == "act":
                    nc.scalar.activation(otm[:, :, 0:3, :], m9v, AF.Copy)
                else:
                    engof("out_m").tensor_copy(otm[:, :, 0:3, :], m9v)
                t3v = t3[:].rearrange("p (f c) -> p c f", c=3)
                if asn["out_t"] == "act":
                    nc.scalar.activation(otm[:, :, 3, :], t3v, AF.Copy)
                else:
                    engof("out_t").tensor_copy(otm[:, :, 3, :], t3v)
                nc.sync.dma_start(out=outv[ti], in_=ot[:])

            def body():
                assert T % 2 == 0
                sts = {}

                def emit_pair_front(p):
                    a, b = 2 * p, 2 * p + 1
                    sts[a] = part_pre_front(a)
                    sts[b] = part_pre_front(b)
                    part_pre_exp(sts[a])
                    part_pre_exp(sts[b])
                    part_trig(sts[a])
                    part_trig(sts[b])

                npairs = T // 2
                order = asn.get("order", "front_mid")
                # pair 0: single-tile chains so mat(0) starts earliest;
                # th2 on DVE (idle anyway), extra table loads hide in fill
                sts[0] = part_pre_front(0, th2_eng="dve")
                part_pre_exp(sts[0])
                part_trig(sts[0])
                sts[1] = part_pre_front(1, th2_eng="dve")
                part_pre_exp(sts[1])
                part_trig(sts[1])
                for p in range(npairs):
                    if order == "front_first":
                        if p + 1 < npairs:
                            emit_pair_front(p + 1)
                        part_mat(2 * p, sts.pop(2 * p))
                        part_mat(2 * p + 1, sts.pop(2 * p + 1))
                    elif order == "front_mid":
                        part_mat(2 * p, sts.pop(2 * p))
                        if p + 1 < npairs:
                            emit_pair_front(p + 1)
                        part_mat(2 * p + 1, sts.pop(2 * p + 1))
                    else:  # mats_then_front
                        part_mat(2 * p, sts.pop(2 * p))
                        part_mat(2 * p + 1, sts.pop(2 * p + 1))
                        if p + 1 < npairs:
                            emit_pair_front(p + 1)

            if loop_rep is None:
                body()
            else:
                with tc.For_i(0, loop_rep, 1, staggered_reset=True):
                    body()

    _split_multi_waits(nc)
    return nc


# ----------------------------------------------------------------------------
# host-side execution
# ----------------------------------------------------------------------------
_CACHE = {}


def _get_runner():
    if "runner" in _CACHE:
        return _CACHE["runner"]
    import jax
    from jax.sharding import Mesh, PartitionSpec
    from jax.experimental.shard_map import shard_map
    from concourse.bass2jax import (
        _bass_exec_p,
        install_neuronx_cc_hook,
        partition_id_tensor,
    )

    nc = build_module()
    install_neuronx_cc_hook()
    partition_name = nc.partition_id_tensor.name if nc.partition_id_tensor else None
    in_names, out_names, out_avals, zero_outs = [], [], [], []
    for alloc in nc.m.functions[0].allocations:
        if not isinstance(alloc, mybir.MemoryLocationSet):
            continue
        name = alloc.memorylocations[0].name
        if alloc.kind == "ExternalInput":
            if name != partition_name:
                in_names.append(name)
        elif alloc.kind == "ExternalOutput":
            shape = tuple(alloc.tensor_shape)
            dtype = mybir.dt.np(alloc.dtype)
            out_names.append(name)
            out_avals.append(jax.core.ShapedArray(shape, dtype))
            zero_outs.append(np.zeros(shape, dtype))
    n_params = len(in_names)
    all_in_names = in_names + out_names + (
        [partition_name] if partition_name else []
    )

    def _body(*args):
        operands = list(args)
        if partition_name is not None:
            operands.append(partition_id_tensor())
        outs = _bass_exec_p.bind(
            *operands,
            out_avals=tuple(out_avals),
            in_names=tuple(all_in_names),
            out_names=tuple(out_names),
            lowering_input_output_aliases=(),
            sim_require_finite=True,
            sim_require_nnan=True,
            nc=nc,
        )
        return tuple(outs)

    devices = jax.devices()[:N_CORES]
    mesh = Mesh(np.asarray(devices), ("core",))
    n_outs = len(out_names)
    jf = jax.jit(
        shard_map(
            _body,
            mesh=mesh,
            in_specs=(PartitionSpec("core"),) * (n_params + n_outs),
            out_specs=(PartitionSpec("core"),) * n_outs,
            check_rep=False,
        ),
        donate_argnums=tuple(range(n_params, n_params + n_outs)),
        keep_unused=True,
    )
    _CACHE["runner"] = (jf, in_names, out_names, zero_outs)
    return _CACHE["runner"]


def kernel(trans, rotat, scal_dir, scal):
    jf, in_names, out_names, zero_outs = _get_runner()
    inputs = {"trans": trans, "rotat": rotat, "scal_dir": scal_dir, "scal": scal}
    # pad to BPAD with ones (zeros would make |v| = 0 -> inf/NaN chains)
    padded = {}
    for k, v in inputs.items():
        a = np.ones((BPAD, 3), dtype=np.float32)
        a[:B] = v
        padded[k] = a
    args = [padded[n] for n in in_names]
    zeros = [np.zeros((N_CORES * z.shape[0], *z.shape[1:]), z.dtype) for z in zero_outs]
    outs = jf(*args, *zeros)
    full = np.asarray(outs[0])  # [BPAD, 12]
    return full[:B].reshape(B, 3, 4).astype(np.float32, copy=False)


if __name__ == "__main__":
    rng = np.random.default_rng(0)
    ins = {
        "trans": rng.normal(size=(B, 3)).astype(np.float32),
        "rotat": rng.normal(size=(B, 3)).astype(np.float32),
        "scal_dir": rng.normal(size=(B, 3)).astype(np.float32),
        "scal": rng.normal(size=(B, 3)).astype(np.float32),
    }
    out = kernel(**ins)
    print(out.shape, out.dtype)


# revision 17
# speedup vs baseline: 1.3408x; 1.0670x over previous
"""AffCoeffToMatrix TRN2 kernel (v2: quarter-angle + 3-engine balance).

For each batch element (B = 2,000,000):
  R = rodrigues(rotat), U = rodrigues(scal_dir), D = exp(scal)
  M = R @ (U @ diag(D) @ U^T);  out = [M | trans]  -> [B, 3, 4] f32

Math per rotation (quaternion form, constants folded into ACT scale/bias):
  v6 = 2*v (deinterleave, ACT scale)        th2 = |v6|^2 = 4 theta^2
  lg = ln(th2); th = exp(.5 lg - ln2) = theta; rth2 = exp(-.5 lg + ln sqrt2)
  s4 = sin(th/4), c4 = sin(th/4 + pi/2)     [no range wrap needed]
  sh2 = 2 s4 c4 = sin(th/2); w2 = sqrt2 cos(th/2) = -2sqrt2 s4^2 + sqrt2
  costh = 1 - 2 sh2^2;  t2 = sh2*rth2;  Q = t2*v6 = sqrt2 sin(th/2)/th * v
  R = costh*I + Q Q^T + [w2*Q]_x
Scaling: e = exp(s/2), W = U diag(e), S = W W^T (6 unique), M = R S.

Sharding: pure batch-parallel over 8 NeuronCores. On-core: [128 part x F free]
planes, fp16 matrix phase, paired (R|U) scalar chain, work split across
DVE / ACT / Pool(gpsimd) engines.
"""
import math
import sys

for _p in ("/opt/trn_rl_repo", "/root/.axon_site/_ro/trn_rl_repo"):
    if _p not in sys.path:
        sys.path.append(_p)

import numpy as np

import concourse.bass as bass
import concourse.mybir as mybir
import concourse.tile as tile

F32 = mybir.dt.float32
F16 = mybir.dt.float16
AF = mybir.ActivationFunctionType
OP = mybir.AluOpType
PI = math.pi
LN2 = math.log(2.0)
LNS2 = math.log(math.sqrt(2.0))
S2 = math.sqrt(2.0)

# ---- hardcoded problem geometry ----
B = 2_000_000
N_CORES = 8
P = 128
F = 328            # free-dim elements per tile
T = 6              # tiles per core
L = F * T          # elements per partition lane
E = P * L          # elements per core
BPAD = N_CORES * E

MAT_DT = F16

# engine assignment for tunable op groups: "dve" | "act" | "pool"
ASSIGN = {
    "sq6": "act",     # squares of v6 (6F)
    "sqq": "act",     # squares of Q (6F)
    "sqw": "act",     # squares of W (9F)
    "sdt": "dve",     # S diag partial add (3F)
    "sdiag": "dve",   # S diag assembly (3F)
    "q3": "dve",      # S offdiag partial add (3F)
    "soff": "dve",    # S offdiag assembly (3F)
    "out_m": "act",   # output interleave copy of m9 (9F)
    "out_t": "act",   # output interleave copy of trans (3F)
    "th2": "dve",     # theta^2 adds (4F)
    "msadd": "dve",   # matmul partial adds (2x9F)
    "shq": "act",     # sh2^2 (2F)
    "order": "mats_then_front",
}


def _split_multi_waits(nc, limit=1, drain_limit=0):
    """This container's walrus cannot encode >1 sync-wait per instruction
    (Drain: none at all). Spill extras onto same-engine NOPs."""
    for b in nc.main_func.blocks:
        new = []
        for ins in b.instructions:
            si = getattr(ins, "sync_info", None)
            waits = list(si.on_wait) if (si is not None and si.on_wait) else []
            lim = drain_limit if isinstance(ins, mybir.InstDrain) else limit
            if len(waits) > lim:
                keep, spill = waits[:lim], waits[lim:]
                for w in spill:
                    nop = mybir.InstNoOp(
                        name=nc.get_next_instruction_name(),
                        sync_info=mybir.SyncInfo(on_wait=[w], on_update=[]),
                        bass_nofuse=True,
                        engine=ins.engine,
                    )
                    nc.register_instruction(nop)
                    new.append(nop)
                ins.sync_info = mybir.SyncInfo(
                    on_wait=keep, on_update=list(si.on_update or [])
                )
            new.append(ins)
        b.instructions[:] = new
    return nc


def build_module(F=F, T=T, mat_dt=MAT_DT, loop_rep=None, assign=None):
    asn = dict(ASSIGN)
    if assign:
        asn.update(assign)
    nc = bass.Bass()
    E_ = P * F * T
    rot = nc.dram_tensor("rotat", [E_, 3], F32, kind="ExternalInput")
    sd = nc.dram_tensor("scal_dir", [E_, 3], F32, kind="ExternalInput")
    sc = nc.dram_tensor("scal", [E_, 3], F32, kind="ExternalInput")
    tr = nc.dram_tensor("trans", [E_, 3], F32, kind="ExternalInput")
    out = nc.dram_tensor("out", [E_, 12], F32, kind="ExternalOutput")

    rotv = rot[:].rearrange("(t p f) c -> t p (f c)", t=T, p=P)
    sdv = sd[:].rearrange("(t p f) c -> t p (f c)", t=T, p=P)
    scv = sc[:].rearrange("(t p f) c -> t p (f c)", t=T, p=P)
    trv = tr[:].rearrange("(t p f) c -> t p (f c)", t=T, p=P)
    outv = out[:].rearrange("(t p f) c -> t p (f c)", t=T, p=P)

    def engof(key):
        return {"dve": nc.vector, "act": nc.scalar, "pool": nc.gpsimd}[asn[key]]

    def square(key, out_ap, in_ap):
        if asn[key] == "act":
            nc.scalar.activation(out_ap, in_ap, AF.Square)
        else:
            engof(key).tensor_mul(out_ap, in_ap, in_ap)

    with tile.TileContext(nc) as tc:
        with (
            tc.tile_pool(name="pc", bufs=1) as pc,
            tc.tile_pool(name="pin", bufs=2) as pin,
            tc.tile_pool(name="pout", bufs=2) as pout,
            tc.tile_pool(name="pch", bufs=2) as pch,
            tc.tile_pool(name="pch2", bufs=4) as pch2,
            tc.tile_pool(name="pm2", bufs=4) as pm2,
            tc.tile_pool(name="pmat", bufs=1) as pmat,
        ):
            npi2 = pc.tile([P, 1], F32, tag="npi2")
            nc.vector.memset(npi2[:], -PI / 2)
            nln2 = pc.tile([P, 1], F32, tag="nln2")
            nc.vector.memset(nln2[:], -LN2)
            lns2 = pc.tile([P, 1], F32, tag="lns2")
            nc.vector.memset(lns2[:], LNS2)

            F2 = 2 * F

            def part_pre_front(ti, th2_eng=None):
                """DMA in + deinterleave + squares + th2 (Pool)."""
                st = {}
                rs6 = pin.tile([P, 6 * F], F32, tag="rs6", name="rs6")
                nc.sync.dma_start(out=rs6[:, : 3 * F], in_=rotv[ti])
                nc.sync.dma_start(out=rs6[:, 3 * F :], in_=sdv[ti])
                c3 = pin.tile([P, 3 * F], F32, tag="sc3", name="sc3")
                nc.sync.dma_start(out=c3[:], in_=scv[ti])
                st["c3"] = c3

                # deinterleave (g f c) -> (g c f), scale by 2:  v6 = 2*v
                v6 = pm2.tile([P, 6 * F], mat_dt, tag="v6", name="v6")
                v6v = v6[:].rearrange("p (g c f) -> p g c f", g=2, c=3)
                rs6v = rs6[:].rearrange("p (g f c) -> p g c f", g=2, c=3)
                nc.scalar.activation(v6v, rs6v, AF.Copy, scale=2.0)
                st["v6"] = v6

                # th2 = |v6|^2  (= 4 theta^2)
                sq6 = pch.tile([P, 6 * F], mat_dt, tag="sq6", name="sq6")
                sq6v = sq6[:].rearrange("p (g c f) -> p g c f", g=2, c=3)
                square("sq6", sq6v, v6v)
                th2a = pch.tile([P, F2], mat_dt, tag="th2a", name="th2a")
                th2av = th2a[:].rearrange("p (g f) -> p g f", g=2)
                eng_th2 = (
                    {"dve": nc.vector, "act": nc.scalar, "pool": nc.gpsimd}[th2_eng]
                    if th2_eng
                    else engof("th2")
                )
                eng_th2.tensor_add(th2av, sq6v[:, :, 0, :], sq6v[:, :, 1, :])
                th2 = pch.tile([P, F2], mat_dt, tag="th2", name="th2")
                th2v = th2[:].rearrange("p (g f) -> p g f", g=2)
                eng_th2.tensor_add(th2v, th2av, sq6v[:, :, 2, :])
                st["th2"] = th2
                return st

            def part_pre_exp(st):
                """natural_log_exp table set chain."""
                th2, c3 = st["th2"], st["c3"]
                lg = pch.tile([P, F2], mat_dt, tag="lg", name="lg")
                nc.scalar.activation(lg[:], th2[:], AF.Ln)
                th = pch2.tile([P, F2], mat_dt, tag="th", name="th")
                nc.scalar.activation(th[:], lg[:], AF.Exp, scale=0.5, bias=nln2[:])
                rth2 = pch2.tile([P, F2], mat_dt, tag="rth2", name="rth2")
                nc.scalar.activation(rth2[:], lg[:], AF.Exp, scale=-0.5, bias=lns2[:])
                e3 = pm2.tile([P, 3 * F], mat_dt, tag="e3", name="e3")
                e3v = e3[:].rearrange("p (c f) -> p c f", c=3)
                nc.scalar.activation(
                    e3v, c3[:].rearrange("p (f c) -> p c f", c=3), AF.Exp, scale=0.5
                )
                st["th"], st["rth2"], st["e3"] = th, rth2, e3

            def part_trig(st):
                """trig set: sh2 = sin(th/2), wn = -cos(th/2); shq filler."""
                th = st["th"]
                sh2 = pch2.tile([P, F2], mat_dt, tag="sh2", name="sh2")
                nc.scalar.activation(sh2[:], th[:], AF.Sin, scale=0.5)
                wn = pch2.tile([P, F2], mat_dt, tag="wn", name="wn")
                nc.scalar.activation(wn[:], th[:], AF.Sin, scale=0.5, bias=npi2[:])
                shq = pch2.tile([P, F2], mat_dt, tag="shq", name="shq")
                square("shq", shq[:], sh2[:])
                st["sh2"], st["wn"], st["shq"] = sh2, wn, shq

            def part_mat(ti, st):
                v6 = st["v6"]
                sh2, wn, shq = st["sh2"], st["wn"], st["shq"]
                rth2, e3 = st["rth2"], st["e3"]
                v6v = v6[:].rearrange("p (g c f) -> p g c f", g=2, c=3)
                e3v = e3[:].rearrange("p (c f) -> p c f", c=3)

                t3 = pin.tile([P, 3 * F], F32, tag="tr3", name="tr3")
                nc.sync.dma_start(out=t3[:], in_=trv[ti])

                def mt(tag, w):
                    return pmat.tile([P, w], mat_dt, tag=tag, name=tag)

                # ---- derived scalars ----
                costh = mt("costh", F2)
                nc.vector.tensor_scalar(costh[:], shq[:], -2.0, 1.0, OP.mult, OP.add)
                t2 = mt("t2", F2)
                nc.vector.tensor_mul(t2[:], sh2[:], rth2[:])
                t2v = t2[:].rearrange("p (g f) -> p g f", g=2)
                wnv = wn[:].rearrange("p (g f) -> p g f", g=2)
                cthv = costh[:].rearrange("p (g f) -> p g f", g=2)

                # ---- Q = t2 * v6;  sqQ on ACT right after ----
                Q6 = mt("Q6", 6 * F)
                Q6v = Q6[:].rearrange("p (g c f) -> p g c f", g=2, c=3)
                nc.vector.tensor_mul(
                    Q6v, t2v.unsqueeze(2).to_broadcast((P, 2, 3, F)), v6v
                )
                sqQ = mt("sqQ", 6 * F)
                sqQv = sqQ[:].rearrange("p (g c f) -> p g c f", g=2, c=3)
                square("sqq", sqQv, Q6v)

                RU18 = mt("RU18", 18 * F)
                ruv = RU18[:].rearrange("p (g k f) -> p g k f", g=2, k=9)
                # p pairs (01, 12, 20); a6n = -sqrt2*cos(th/2)*Q aligned (z, x, y)
                p6 = mt("p6", 6 * F)
                p6v = p6[:].rearrange("p (g j f) -> p g j f", g=2, j=3)
                nc.vector.tensor_mul(
                    p6v[:, :, 0:2, :], Q6v[:, :, 0:2, :], Q6v[:, :, 1:3, :]
                )
                nc.vector.tensor_mul(p6v[:, :, 2, :], Q6v[:, :, 2, :], Q6v[:, :, 0, :])
                a6 = mt("a6", 6 * F)
                a6v = a6[:].rearrange("p (g j f) -> p g j f", g=2, j=3)
                nc.vector.scalar_tensor_tensor(
                    a6v[:, :, 0, :], Q6v[:, :, 2, :], S2, wnv, OP.mult, OP.mult
                )
                nc.vector.scalar_tensor_tensor(
                    a6v[:, :, 1, :], Q6v[:, :, 0, :], S2, wnv, OP.mult, OP.mult
                )
                nc.vector.scalar_tensor_tensor(
                    a6v[:, :, 2, :], Q6v[:, :, 1, :], S2, wnv, OP.mult, OP.mult
                )
                # a6 holds NEGATED a-terms -> swap add/sub:
                # plus(@3,@7,@2) = p - a6n;  minus(@1,@5,@6) = p + a6n
                nc.vector.tensor_sub(
                    ruv[:, :, 3:8:4, :], p6v[:, :, 0:2, :], a6v[:, :, 0:2, :]
                )
                nc.vector.tensor_sub(ruv[:, :, 2, :], p6v[:, :, 2, :], a6v[:, :, 2, :])
                nc.vector.tensor_add(
                    ruv[:, :, 1:6:4, :], p6v[:, :, 0:2, :], a6v[:, :, 0:2, :]
                )
                nc.vector.tensor_add(ruv[:, :, 6, :], p6v[:, :, 2, :], a6v[:, :, 2, :])
                # diag @ (0,4,8) = sqQ + costh  (ACT had time for sqQ by now)
                nc.vector.tensor_add(
                    ruv[:, :, 0:9:4, :],
                    sqQv,
                    cthv.unsqueeze(2).to_broadcast((P, 2, 3, F)),
                )
                R9v = RU18[:, : 9 * F].rearrange("p (k f) -> p k f", k=9)
                U9v = RU18[:, 9 * F :].rearrange("p (i k f) -> p i k f", i=3, k=3)

                # ---- W = U * diag(e);  sqW on ACT; S = W W^T ----
                W9 = mt("W9", 9 * F)
                W9v4 = W9[:].rearrange("p (i k f) -> p i k f", i=3, k=3)
                e_b = e3v.unsqueeze(1).to_broadcast((P, 3, 3, F))
                nc.vector.tensor_mul(W9v4, U9v, e_b)
                W9v = W9[:].rearrange("p (k f) -> p k f", k=9)
                sqW = mt("sqW", 9 * F)
                square("sqw", sqW[:], W9[:])
                sqWv = sqW[:].rearrange("p (i k f) -> p i k f", i=3, k=3)
                # off-diagonal first (gives ACT time for sqW)
                pp = mt("pp", 9 * F)
                ppv = pp[:].rearrange("p (g k f) -> p g k f", g=3, k=3)
                w0b = W9v[:, 0:3, :].unsqueeze(1).to_broadcast((P, 2, 3, F))
                nc.vector.tensor_mul(ppv[:, 0:2, :, :], w0b, W9v4[:, 1:3, :, :])
                nc.vector.tensor_mul(ppv[:, 2, :, :], W9v[:, 3:6, :], W9v[:, 6:9, :])
                # S unique-6 with holes: S00@0 S01@1 S02@2 S11@3 S12@5 S22@8
                S9 = mt("S9", 9 * F)
                S9v = S9[:].rearrange("p (k f) -> p k f", k=9)
                q3 = mt("q3", 3 * F)
                q3v = q3[:].rearrange("p (g f) -> p g f", g=3)
                engof("q3").tensor_add(q3v, ppv[:, :, 0, :], ppv[:, :, 1, :])
                eng_soff = engof("soff")
                eng_soff.tensor_add(
                    S9v[:, 1:3, :], q3v[:, 0:2, :], ppv[:, 0:2, 2, :]
                )
                eng_soff.tensor_add(S9v[:, 5, :], q3v[:, 2, :], ppv[:, 2, 2, :])
                sdt = mt("sdt", 3 * F)
                sdtv = sdt[:].rearrange("p (c f) -> p c f", c=3)
                engof("sdt").tensor_add(sdtv, sqWv[:, :, 0, :], sqWv[:, :, 1, :])
                eng_sdiag = engof("sdiag")
                eng_sdiag.tensor_add(
                    S9v[:, 0:4:3, :], sdtv[:, 0:2, :], sqWv[:, 0:2, 2, :]
                )
                eng_sdiag.tensor_add(S9v[:, 8, :], sdtv[:, 2, :], sqWv[:, 2, 2, :])

                # ---- M = R @ S  (5 wide instructions) ----
                srow = [
                    S9v[:, 0:3, :].unsqueeze(1).to_broadcast((P, 3, 3, F)),
                    S9v[:, 1:7:2, :].unsqueeze(1).to_broadcast((P, 3, 3, F)),
                    S9v[:, 2:9:3, :].unsqueeze(1).to_broadcast((P, 3, 3, F)),
                ]

                def rcol(k):
                    return (
                        R9v[:, k:9:3, :]
                        .unsqueeze(2)
                        .to_broadcast((P, 3, 3, F))
                    )

                mp1 = mt("mp1", 9 * F)
                mp1v = mp1[:].rearrange("p (r k f) -> p r k f", r=3, k=3)
                nc.vector.tensor_mul(mp1v, rcol(0), srow[0])
                mp2 = mt("mp2", 9 * F)
                mp2v = mp2[:].rearrange("p (r k f) -> p r k f", r=3, k=3)
                nc.vector.tensor_mul(mp2v, rcol(1), srow[1])
                eng_ms = engof("msadd")
                eng_ms.tensor_add(mp1v, mp1v, mp2v)  # ms in-place on mp1
                mp3 = mt("mp3", 9 * F)
                mp3v = mp3[:].rearrange("p (r k f) -> p r k f", r=3, k=3)
                nc.vector.tensor_mul(mp3v, rcol(2), srow[2])
                m9 = pmat.tile([P, 9 * F], mat_dt, tag="m9", name="m9")
                m9v = m9[:].rearrange("p (r k f) -> p r k f", r=3, k=3)
                eng_ms.tensor_add(m9v, mp1v, mp3v)

                # ---- interleave to f32 out: rows + trans ----
                ot = pout.tile([P, 12 * F], F32, tag="out", name="ot")
                otm = ot[:].rearrange("p (f r k) -> p r k f", r=3, k=4)
                if asn["out_m"] == "act":
                    nc.scalar.activation(otm[:, :, 0:3, :], m9v, AF.Copy)
                else:
                    engof("out_m").tensor_copy(otm[:, :, 0:3, :], m9v)
                t3v = t3[:].rearrange("p (f c) -> p c f", c=3)
                if asn["out_t"] == "act":
                    nc.scalar.activation(otm[:, :, 3, :], t3v, AF.Copy)
                else:
                    engof("out_t").tensor_copy(otm[:, :, 3, :], t3v)
                nc.sync.dma_start(out=outv[ti], in_=ot[:])

            def body():
                assert T % 2 == 0
                sts = {}

                def emit_pair_front(p):
                    a, b = 2 * p, 2 * p + 1
                    sts[a] = part_pre_front(a)
                    sts[b] = part_pre_front(b)
                    part_pre_exp(sts[a])
                    part_pre_exp(sts[b])
                    part_trig(sts[a])
                    part_trig(sts[b])

                npairs = T // 2
                order = asn.get("order", "front_mid")
                # pair 0: single-tile chains so mat(0) starts earliest;
                # th2 on DVE (idle anyway), extra table loads hide in fill
                sts[0] = part_pre_front(0, th2_eng="dve")
                part_pre_exp(sts[0])
                part_trig(sts[0])
                sts[1] = part_pre_front(1, th2_eng="dve")
                part_pre_exp(sts[1])
                part_trig(sts[1])
                for p in range(npairs):
                    if order == "front_first":
                        if p + 1 < npairs:
                            emit_pair_front(p + 1)
                        part_mat(2 * p, sts.pop(2 * p))
                        part_mat(2 * p + 1, sts.pop(2 * p + 1))
                    elif order == "front_mid":
                        part_mat(2 * p, sts.pop(2 * p))
                        if p + 1 < npairs:
                            emit_pair_front(p + 1)
                        part_mat(2 * p + 1, sts.pop(2 * p + 1))
                    else:  # mats_then_front
                        part_mat(2 * p, sts.pop(2 * p))
                        part_mat(2 * p + 1, sts.pop(2 * p + 1))
                        if p + 1 < npairs:
                            emit_pair_front(p + 1)

            if loop_rep is None:
                body()
            else:
                with tc.For_i(0, loop_rep, 1, staggered_reset=True):
                    body()

    _split_multi_waits(nc)
    return nc


# ----------------------------------------------------------------------------
# host-side execution
# ----------------------------------------------------------------------------
_CACHE = {}


def _get_runner():
    if "runner" in _CACHE:
        return _CACHE["runner"]
    import jax
    from jax.sharding import Mesh, PartitionSpec
    from jax.experimental.shard_map import shard_map
    from concourse.bass2jax import (
        _bass_exec_p,
        install_neuronx_cc_hook,
        partition_id_tensor,
    )

    nc = build_module()
    install_neuronx_cc_hook()
    partition_name = nc.partition_id_tensor.name if nc.partition_id_tensor else None
    in_names, out_names, out_avals, zero_outs = [], [], [], []
    for alloc in nc.m.functions[0].allocations:
        if not isinstance(alloc, mybir.MemoryLocationSet):
            continue
        name = alloc.memorylocations[0].name
        if alloc.kind == "ExternalInput":
            if name != partition_name:
                in_names.append(name)
        elif alloc.kind == "ExternalOutput":
            shape = tuple(alloc.tensor_shape)
            dtype = mybir.dt.np(alloc.dtype)
            out_names.append(name)
            out_avals.append(jax.core.ShapedArray(shape, dtype))
            zero_outs.append(np.zeros(shape, dtype))
    n_params = len(in_names)
    all_in_names = in_names + out_names + (
        [partition_name] if partition_name else []
    )

    def _body(*args):
        operands = list(args)
        if partition_name is not None:
            operands.append(partition_id_tensor())
        outs = _bass_exec_p.bind(
            *operands,
            out_avals=tuple(out_avals),
            in_names=tuple(all_in_names),
            out_names=tuple(out_names),
            lowering_input_output_aliases=(),
            sim_require_finite=True,
            sim_require_nnan=True,
            nc=nc,
        )
        return tuple(outs)

    devices = jax.devices()[:N_CORES]
    mesh = Mesh(np.asarray(devices), ("core",))
    n_outs = len(out_names)
    jf = jax.jit(
        shard_map(
            _body,
            mesh=mesh,
            in_specs=(PartitionSpec("core"),) * (n_params + n_outs),
            out_specs=(PartitionSpec("core"),) * n_outs,
            check_rep=False,
        ),
        donate_argnums=tuple(range(n_params, n_params + n_outs)),
        keep_unused=True,
    )
    _CACHE["runner"] = (jf, in_names, out_names, zero_outs)
    return _CACHE["runner"]


def kernel(trans, rotat, scal_dir, scal):
    jf, in_names, out_names, zero_outs = _get_runner()
    inputs = {"trans": trans, "rotat": rotat, "scal_dir": scal_dir, "scal": scal}
    # pad to BPAD with ones (zeros would make |v| = 0 -> inf/NaN chains)
    padded = {}
    for k, v in inputs.items():
        a = np.ones((BPAD, 3), dtype=np.float32)
        a[:B] = v
        padded[k] = a
    args = [padded[n] for n in in_names]
    zeros = [np.zeros((N_CORES * z.shape[0], *z.shape[1:]), z.dtype) for z in zero_outs]
    outs = jf(*args, *zeros)
    full = np.asarray(outs[0])  # [BPAD, 12]
    return full[:B].reshape(B, 3, 4).astype(np.float32, copy=False)


if __name__ == "__main__":
    rng = np.random.default_rng(0)
    ins = {
        "trans": rng.normal(size=(B, 3)).astype(np.float32),
        "rotat": rng.normal(size=(B, 3)).astype(np.float32),
        "scal_dir": rng.normal(size=(B, 3)).astype(np.float32),
        "scal": rng.normal(size=(B, 3)).astype(np.float32),
    }
    out = kernel(**ins)
    print(out.shape, out.dtype)


# revision 19
# speedup vs baseline: 1.3709x; 1.0225x over previous
"""AffCoeffToMatrix TRN2 kernel (v2: quarter-angle + 3-engine balance).

For each batch element (B = 2,000,000):
  R = rodrigues(rotat), U = rodrigues(scal_dir), D = exp(scal)
  M = R @ (U @ diag(D) @ U^T);  out = [M | trans]  -> [B, 3, 4] f32

Math per rotation (quaternion form, constants folded into ACT scale/bias):
  v6 = 2*v (deinterleave, ACT scale)        th2 = |v6|^2 = 4 theta^2
  lg = ln(th2); th = exp(.5 lg - ln2) = theta; rth2 = exp(-.5 lg + ln sqrt2)
  s4 = sin(th/4), c4 = sin(th/4 + pi/2)     [no range wrap needed]
  sh2 = 2 s4 c4 = sin(th/2); w2 = sqrt2 cos(th/2) = -2sqrt2 s4^2 + sqrt2
  costh = 1 - 2 sh2^2;  t2 = sh2*rth2;  Q = t2*v6 = sqrt2 sin(th/2)/th * v
  R = costh*I + Q Q^T + [w2*Q]_x
Scaling: e = exp(s/2), W = U diag(e), S = W W^T (6 unique), M = R S.

Sharding: pure batch-parallel over 8 NeuronCores. On-core: [128 part x F free]
planes, fp16 matrix phase, paired (R|U) scalar chain, work split across
DVE / ACT / Pool(gpsimd) engines.
"""
import math
import sys

for _p in ("/opt/trn_rl_repo", "/root/.axon_site/_ro/trn_rl_repo"):
    if _p not in sys.path:
        sys.path.append(_p)

import numpy as np

import concourse.bass as bass
import concourse.mybir as mybir
import concourse.tile as tile

F32 = mybir.dt.float32
F16 = mybir.dt.float16
AF = mybir.ActivationFunctionType
OP = mybir.AluOpType
PI = math.pi
LN2 = math.log(2.0)
LNS2 = math.log(math.sqrt(2.0))
S2 = math.sqrt(2.0)

# ---- hardcoded problem geometry ----
B = 2_000_000
N_CORES = 8
P = 128
F = 328            # free-dim elements per tile
T = 6              # tiles per core
L = F * T          # elements per partition lane
E = P * L          # elements per core
BPAD = N_CORES * E

MAT_DT = F16

# engine assignment for tunable op groups: "dve" | "act" | "pool"
ASSIGN = {
    "sq6": "act",     # squares of v6 (6F)
    "sqq": "act",     # squares of Q (6F)
    "sqw": "act",     # squares of W (9F)
    "sdt": "dve",     # S diag partial add (3F)
    "sdiag": "dve",   # S diag assembly (3F)
    "q3": "dve",      # S offdiag partial add (3F)
    "soff": "dve",    # S offdiag assembly (3F)
    "out_m": "act",   # output interleave copy of m9 (9F)
    "out_t": "act",   # output interleave copy of trans (3F)
    "th2": "dve",     # theta^2 adds (4F)
    "msadd": "dve",   # matmul partial adds (2x9F)
    "shq": "act",     # sh2^2 (2F)
    "order": "mats_then_front",
}


def _split_multi_waits(nc, limit=1, drain_limit=0):
    """This container's walrus cannot encode >1 sync-wait per instruction
    (Drain: none at all). Spill extras onto same-engine NOPs."""
    for b in nc.main_func.blocks:
        new = []
        for ins in b.instructions:
            si = getattr(ins, "sync_info", None)
            waits = list(si.on_wait) if (si is not None and si.on_wait) else []
            lim = drain_limit if isinstance(ins, mybir.InstDrain) else limit
            if len(waits) > lim:
                keep, spill = waits[:lim], waits[lim:]
                for w in spill:
                    nop = mybir.InstNoOp(
                        name=nc.get_next_instruction_name(),
                        sync_info=mybir.SyncInfo(on_wait=[w], on_update=[]),
                        bass_nofuse=True,
                        engine=ins.engine,
                    )
                    nc.register_instruction(nop)
                    new.append(nop)
                ins.sync_info = mybir.SyncInfo(
                    on_wait=keep, on_update=list(si.on_update or [])
                )
            new.append(ins)
        b.instructions[:] = new
    return nc


def build_module(F=F, T=T, mat_dt=MAT_DT, loop_rep=None, assign=None):
    asn = dict(ASSIGN)
    if assign:
        asn.update(assign)
    nc = bass.Bass()
    E_ = P * F * T
    rot = nc.dram_tensor("rotat", [E_, 3], F32, kind="ExternalInput")
    sd = nc.dram_tensor("scal_dir", [E_, 3], F32, kind="ExternalInput")
    sc = nc.dram_tensor("scal", [E_, 3], F32, kind="ExternalInput")
    tr = nc.dram_tensor("trans", [E_, 3], F32, kind="ExternalInput")
    out = nc.dram_tensor("out", [E_, 12], F32, kind="ExternalOutput")

    rotv = rot[:].rearrange("(t p f) c -> t p (f c)", t=T, p=P)
    sdv = sd[:].rearrange("(t p f) c -> t p (f c)", t=T, p=P)
    scv = sc[:].rearrange("(t p f) c -> t p (f c)", t=T, p=P)
    trv = tr[:].rearrange("(t p f) c -> t p (f c)", t=T, p=P)
    outv = out[:].rearrange("(t p f) c -> t p (f c)", t=T, p=P)

    def engof(key):
        return {"dve": nc.vector, "act": nc.scalar, "pool": nc.gpsimd}[asn[key]]

    def square(key, out_ap, in_ap):
        if asn[key] == "act":
            nc.scalar.activation(out_ap, in_ap, AF.Square)
        else:
            engof(key).tensor_mul(out_ap, in_ap, in_ap)

    with tile.TileContext(nc) as tc:
        with (
            tc.tile_pool(name="pc", bufs=1) as pc,
            tc.tile_pool(name="pin", bufs=2) as pin,
            tc.tile_pool(name="pout", bufs=2) as pout,
            tc.tile_pool(name="pch", bufs=2) as pch,
            tc.tile_pool(name="pch2", bufs=4) as pch2,
            tc.tile_pool(name="pm2", bufs=4) as pm2,
            tc.tile_pool(name="pmat", bufs=1) as pmat,
        ):
            npi2 = pc.tile([P, 1], F32, tag="npi2")
            nc.vector.memset(npi2[:], -PI / 2)
            nln2 = pc.tile([P, 1], F32, tag="nln2")
            nc.vector.memset(nln2[:], -LN2)
            lns2 = pc.tile([P, 1], F32, tag="lns2")
            nc.vector.memset(lns2[:], LNS2)

            F2 = 2 * F

            def part_pre_front(ti, th2_eng=None):
                """DMA in + deinterleave + squares + th2 (Pool)."""
                st = {}
                rs6 = pin.tile([P, 6 * F], F32, tag="rs6", name="rs6")
                nc.sync.dma_start(out=rs6[:, : 3 * F], in_=rotv[ti])
                nc.sync.dma_start(out=rs6[:, 3 * F :], in_=sdv[ti])
                c3 = pin.tile([P, 3 * F], F32, tag="sc3", name="sc3")
                nc.sync.dma_start(out=c3[:], in_=scv[ti])
                st["c3"] = c3

                # deinterleave (g f c) -> (g c f), scale by 2:  v6 = 2*v
                v6 = pm2.tile([P, 6 * F], mat_dt, tag="v6", name="v6")
                v6v = v6[:].rearrange("p (g c f) -> p g c f", g=2, c=3)
                rs6v = rs6[:].rearrange("p (g f c) -> p g c f", g=2, c=3)
                for g in range(2):
                    nc.scalar.activation(v6v[:, g], rs6v[:, g], AF.Copy, scale=2.0)
                st["v6"] = v6

                # th2 = |v6|^2  (= 4 theta^2)
                sq6 = pch.tile([P, 6 * F], mat_dt, tag="sq6", name="sq6")
                sq6v = sq6[:].rearrange("p (g c f) -> p g c f", g=2, c=3)
                for g in range(2):
                    square("sq6", sq6v[:, g], v6v[:, g])
                th2a = pch.tile([P, F2], mat_dt, tag="th2a", name="th2a")
                th2av = th2a[:].rearrange("p (g f) -> p g f", g=2)
                eng_th2 = (
                    {"dve": nc.vector, "act": nc.scalar, "pool": nc.gpsimd}[th2_eng]
                    if th2_eng
                    else engof("th2")
                )
                eng_th2.tensor_add(th2av, sq6v[:, :, 0, :], sq6v[:, :, 1, :])
                th2 = pch.tile([P, F2], mat_dt, tag="th2", name="th2")
                th2v = th2[:].rearrange("p (g f) -> p g f", g=2)
                eng_th2.tensor_add(th2v, th2av, sq6v[:, :, 2, :])
                st["th2"] = th2
                return st

            def part_pre_exp(st):
                """natural_log_exp table set chain."""
                th2, c3 = st["th2"], st["c3"]
                lg = pch.tile([P, F2], mat_dt, tag="lg", name="lg")
                nc.scalar.activation(lg[:], th2[:], AF.Ln)
                th = pch2.tile([P, F2], mat_dt, tag="th", name="th")
                nc.scalar.activation(th[:], lg[:], AF.Exp, scale=0.5, bias=nln2[:])
                rth2 = pch2.tile([P, F2], mat_dt, tag="rth2", name="rth2")
                nc.scalar.activation(rth2[:], lg[:], AF.Exp, scale=-0.5, bias=lns2[:])
                e3 = pm2.tile([P, 3 * F], mat_dt, tag="e3", name="e3")
                e3v = e3[:].rearrange("p (c f) -> p c f", c=3)
                nc.scalar.activation(
                    e3v, c3[:].rearrange("p (f c) -> p c f", c=3), AF.Exp, scale=0.5
                )
                st["th"], st["rth2"], st["e3"] = th, rth2, e3

            def part_trig(st):
                """trig set: sh2 = sin(th/2), wn = -cos(th/2); shq filler."""
                th = st["th"]
                sh2 = pch2.tile([P, F2], mat_dt, tag="sh2", name="sh2")
                nc.scalar.activation(sh2[:], th[:], AF.Sin, scale=0.5)
                wn = pch2.tile([P, F2], mat_dt, tag="wn", name="wn")
                nc.scalar.activation(wn[:], th[:], AF.Sin, scale=0.5, bias=npi2[:])
                shq = pch2.tile([P, F2], mat_dt, tag="shq", name="shq")
                square("shq", shq[:], sh2[:])
                st["sh2"], st["wn"], st["shq"] = sh2, wn, shq

            def part_mat(ti, st):
                v6 = st["v6"]
                sh2, wn, shq = st["sh2"], st["wn"], st["shq"]
                rth2, e3 = st["rth2"], st["e3"]
                v6v = v6[:].rearrange("p (g c f) -> p g c f", g=2, c=3)
                e3v = e3[:].rearrange("p (c f) -> p c f", c=3)

                t3 = pin.tile([P, 3 * F], F32, tag="tr3", name="tr3")
                nc.sync.dma_start(out=t3[:], in_=trv[ti])

                def mt(tag, w):
                    return pmat.tile([P, w], mat_dt, tag=tag, name=tag)

                # ---- derived scalars ----
                costh = mt("costh", F2)
                nc.vector.tensor_scalar(costh[:], shq[:], -2.0, 1.0, OP.mult, OP.add)
                t2 = mt("t2", F2)
                nc.vector.tensor_mul(t2[:], sh2[:], rth2[:])
                t2v = t2[:].rearrange("p (g f) -> p g f", g=2)
                wnv = wn[:].rearrange("p (g f) -> p g f", g=2)
                cthv = costh[:].rearrange("p (g f) -> p g f", g=2)

                # ---- Q = t2 * v6;  sqQ on ACT right after (3D per g) ----
                Q6 = mt("Q6", 6 * F)
                Q6v = Q6[:].rearrange("p (g c f) -> p g c f", g=2, c=3)
                for g in range(2):
                    nc.vector.tensor_mul(
                        Q6v[:, g],
                        t2v[:, g, :].unsqueeze(1).to_broadcast((P, 3, F)),
                        v6v[:, g],
                    )
                sqQ = mt("sqQ", 6 * F)
                sqQv = sqQ[:].rearrange("p (g c f) -> p g c f", g=2, c=3)
                for g in range(2):
                    square("sqq", sqQv[:, g], Q6v[:, g])

                RU18 = mt("RU18", 18 * F)
                ruv = RU18[:].rearrange("p (g k f) -> p g k f", g=2, k=9)
                # p pairs (01, 12, 20); a6n = -sqrt2*cos(th/2)*Q aligned (z, x, y)
                p6 = mt("p6", 6 * F)
                p6v = p6[:].rearrange("p (g j f) -> p g j f", g=2, j=3)
                for g in range(2):
                    nc.vector.tensor_mul(
                        p6v[:, g, 0:2, :], Q6v[:, g, 0:2, :], Q6v[:, g, 1:3, :]
                    )
                nc.vector.tensor_mul(p6v[:, :, 2, :], Q6v[:, :, 2, :], Q6v[:, :, 0, :])
                a6 = mt("a6", 6 * F)
                a6v = a6[:].rearrange("p (g j f) -> p g j f", g=2, j=3)
                nc.vector.scalar_tensor_tensor(
                    a6v[:, :, 0, :], Q6v[:, :, 2, :], S2, wnv, OP.mult, OP.mult
                )
                nc.vector.scalar_tensor_tensor(
                    a6v[:, :, 1, :], Q6v[:, :, 0, :], S2, wnv, OP.mult, OP.mult
                )
                nc.vector.scalar_tensor_tensor(
                    a6v[:, :, 2, :], Q6v[:, :, 1, :], S2, wnv, OP.mult, OP.mult
                )
                # a6 holds NEGATED a-terms -> swap add/sub:
                # plus(@3,@7,@2) = p - a6n;  minus(@1,@5,@6) = p + a6n
                for g in range(2):
                    nc.vector.tensor_sub(
                        ruv[:, g, 3:8:4, :], p6v[:, g, 0:2, :], a6v[:, g, 0:2, :]
                    )
                    nc.vector.tensor_add(
                        ruv[:, g, 1:6:4, :], p6v[:, g, 0:2, :], a6v[:, g, 0:2, :]
                    )
                nc.vector.tensor_sub(ruv[:, :, 2, :], p6v[:, :, 2, :], a6v[:, :, 2, :])
                nc.vector.tensor_add(ruv[:, :, 6, :], p6v[:, :, 2, :], a6v[:, :, 2, :])
                # diag @ (0,4,8) = sqQ + costh
                for g in range(2):
                    nc.vector.tensor_add(
                        ruv[:, g, 0:9:4, :],
                        sqQv[:, g],
                        cthv[:, g, :].unsqueeze(1).to_broadcast((P, 3, F)),
                    )
                R9v = RU18[:, : 9 * F].rearrange("p (k f) -> p k f", k=9)
                U9v = RU18[:, 9 * F :].rearrange("p (i k f) -> p i k f", i=3, k=3)

                # ---- W = U * diag(e) (3D per row);  sqW on ACT; S = W W^T ----
                W9 = mt("W9", 9 * F)
                W9v4 = W9[:].rearrange("p (i k f) -> p i k f", i=3, k=3)
                for i in range(3):
                    nc.vector.tensor_mul(W9v4[:, i], U9v[:, i], e3v)
                W9v = W9[:].rearrange("p (k f) -> p k f", k=9)
                sqW = mt("sqW", 9 * F)
                square("sqw", sqW[:], W9[:])
                sqWv = sqW[:].rearrange("p (i k f) -> p i k f", i=3, k=3)
                # off-diagonal products (3D, no broadcast)
                pp = mt("pp", 9 * F)
                ppv = pp[:].rearrange("p (g k f) -> p g k f", g=3, k=3)
                nc.vector.tensor_mul(ppv[:, 0], W9v[:, 0:3, :], W9v[:, 3:6, :])
                nc.vector.tensor_mul(ppv[:, 1], W9v[:, 0:3, :], W9v[:, 6:9, :])
                nc.vector.tensor_mul(ppv[:, 2], W9v[:, 3:6, :], W9v[:, 6:9, :])
                # S unique-6 with holes: S00@0 S01@1 S02@2 S11@3 S12@5 S22@8
                S9 = mt("S9", 9 * F)
                S9v = S9[:].rearrange("p (k f) -> p k f", k=9)
                q3 = mt("q3", 3 * F)
                q3v = q3[:].rearrange("p (g f) -> p g f", g=3)
                engof("q3").tensor_add(q3v, ppv[:, :, 0, :], ppv[:, :, 1, :])
                eng_soff = engof("soff")
                eng_soff.tensor_add(
                    S9v[:, 1:3, :], q3v[:, 0:2, :], ppv[:, 0:2, 2, :]
                )
                eng_soff.tensor_add(S9v[:, 5, :], q3v[:, 2, :], ppv[:, 2, 2, :])
                sdt = mt("sdt", 3 * F)
                sdtv = sdt[:].rearrange("p (c f) -> p c f", c=3)
                engof("sdt").tensor_add(sdtv, sqWv[:, :, 0, :], sqWv[:, :, 1, :])
                eng_sdiag = engof("sdiag")
                eng_sdiag.tensor_add(
                    S9v[:, 0:4:3, :], sdtv[:, 0:2, :], sqWv[:, 0:2, 2, :]
                )
                eng_sdiag.tensor_add(S9v[:, 8, :], sdtv[:, 2, :], sqWv[:, 2, 2, :])

                # ---- M = R @ S  (3D per-row instructions) ----
                srow3 = [
                    S9v[:, 0:3, :],
                    S9v[:, 1:7:2, :],
                    S9v[:, 2:9:3, :],
                ]

                def bca(ap_pf):
                    return ap_pf.unsqueeze(1).to_broadcast((P, 3, F))

                m9 = pmat.tile([P, 9 * F], mat_dt, tag="m9", name="m9")
                m9v = m9[:].rearrange("p (r k f) -> p r k f", r=3, k=3)
                for i in range(3):
                    mp1 = mt("mp1", 3 * F)
                    mp1v = mp1[:].rearrange("p (c f) -> p c f", c=3)
                    nc.vector.tensor_mul(mp1v, bca(R9v[:, 3 * i, :]), srow3[0])
                    mp2 = mt("mp2", 3 * F)
                    mp2v = mp2[:].rearrange("p (c f) -> p c f", c=3)
                    nc.vector.tensor_mul(mp2v, bca(R9v[:, 3 * i + 1, :]), srow3[1])
                    ms = mt("ms", 3 * F)
                    msv = ms[:].rearrange("p (c f) -> p c f", c=3)
                    nc.vector.tensor_add(msv, mp1v, mp2v)
                    mp3 = mt("mp3", 3 * F)
                    mp3v = mp3[:].rearrange("p (c f) -> p c f", c=3)
                    nc.vector.tensor_mul(mp3v, bca(R9v[:, 3 * i + 2, :]), srow3[2])
                    nc.vector.tensor_add(m9v[:, i, :, :], msv, mp3v)

                # ---- interleave to f32 out: rows + trans (3D per row) ----
                ot = pout.tile([P, 12 * F], F32, tag="out", name="ot")
                otm = ot[:].rearrange("p (f r k) -> p r k f", r=3, k=4)
                for r in range(3):
                    if asn["out_m"] == "act":
                        nc.scalar.activation(otm[:, r, 0:3, :], m9v[:, r], AF.Copy)
                    else:
                        engof("out_m").tensor_copy(otm[:, r, 0:3, :], m9v[:, r])
                t3v = t3[:].rearrange("p (f c) -> p c f", c=3)
                if asn["out_t"] == "act":
                    nc.scalar.activation(otm[:, :, 3, :], t3v, AF.Copy)
                else:
                    engof("out_t").tensor_copy(otm[:, :, 3, :], t3v)
                nc.sync.dma_start(out=outv[ti], in_=ot[:])

            def body():
                assert T % 2 == 0
                sts = {}

                def emit_pair_front(p):
                    a, b = 2 * p, 2 * p + 1
                    sts[a] = part_pre_front(a)
                    sts[b] = part_pre_front(b)
                    part_pre_exp(sts[a])
                    part_pre_exp(sts[b])
                    part_trig(sts[a])
                    part_trig(sts[b])

                npairs = T // 2
                order = asn.get("order", "front_mid")
                # pair 0: single-tile chains so mat(0) starts earliest;
                # th2 on DVE (idle anyway), extra table loads hide in fill
                sts[0] = part_pre_front(0, th2_eng="dve")
                part_pre_exp(sts[0])
                part_trig(sts[0])
                sts[1] = part_pre_front(1, th2_eng="dve")
                part_pre_exp(sts[1])
                part_trig(sts[1])
                for p in range(npairs):
                    if order == "front_first":
                        if p + 1 < npairs:
                            emit_pair_front(p + 1)
                        part_mat(2 * p, sts.pop(2 * p))
                        part_mat(2 * p + 1, sts.pop(2 * p + 1))
                    elif order == "front_mid":
                        part_mat(2 * p, sts.pop(2 * p))
                        if p + 1 < npairs:
                            emit_pair_front(p + 1)
                        part_mat(2 * p + 1, sts.pop(2 * p + 1))
                    else:  # mats_then_front
                        part_mat(2 * p, sts.pop(2 * p))
                        part_mat(2 * p + 1, sts.pop(2 * p + 1))
                        if p + 1 < npairs:
                            emit_pair_front(p + 1)

            if loop_rep is None:
                body()
            else:
                with tc.For_i(0, loop_rep, 1, staggered_reset=True):
                    body()

    _split_multi_waits(nc)
    return nc


# ----------------------------------------------------------------------------
# host-side execution
# ----------------------------------------------------------------------------
_CACHE = {}


def _get_runner():
    if "runner" in _CACHE:
        return _CACHE["runner"]
    import jax
    from jax.sharding import Mesh, PartitionSpec
    from jax.experimental.shard_map import shard_map
    from concourse.bass2jax import (
        _bass_exec_p,
        install_neuronx_cc_hook,
        partition_id_tensor,
    )

    nc = build_module()
    install_neuronx_cc_hook()
    partition_name = nc.partition_id_tensor.name if nc.partition_id_tensor else None
    in_names, out_names, out_avals, zero_outs = [], [], [], []
    for alloc in nc.m.functions[0].allocations:
        if not isinstance(alloc, mybir.MemoryLocationSet):
            continue
        name = alloc.memorylocations[0].name
        if alloc.kind == "ExternalInput":
            if name != partition_name:
                in_names.append(name)
        elif alloc.kind == "ExternalOutput":
            shape = tuple(alloc.tensor_shape)
            dtype = mybir.dt.np(alloc.dtype)
            out_names.append(name)
            out_avals.append(jax.core.ShapedArray(shape, dtype))
            zero_outs.append(np.zeros(shape, dtype))
    n_params = len(in_names)
    all_in_names = in_names + out_names + (
        [partition_name] if partition_name else []
    )

    def _body(*args):
        operands = list(args)
        if partition_name is not None:
            operands.append(partition_id_tensor())
        outs = _bass_exec_p.bind(
            *operands,
            out_avals=tuple(out_avals),
            in_names=tuple(all_in_names),
            out_names=tuple(out_names),
            lowering_input_output_aliases=(),
            sim_require_finite=True,
            sim_require_nnan=True,
            nc=nc,
        )
        return tuple(outs)

    devices = jax.devices()[:N_CORES]
    mesh = Mesh(np.asarray(devices), ("core",))
    n_outs = len(out_names)
    jf = jax.jit(
        shard_map(
            _body,
            mesh=mesh,
            in_specs=(PartitionSpec("core"),) * (n_params + n_outs),
            out_specs=(PartitionSpec("core"),) * n_outs,
            check_rep=False,
        ),
        donate_argnums=tuple(range(n_params, n_params + n_outs)),
        keep_unused=True,
    )
    _CACHE["runner"] = (jf, in_names, out_names, zero_outs)
    return _CACHE["runner"]


def kernel(trans, rotat, scal_dir, scal):
    jf, in_names, out_names, zero_outs = _get_runner()
    inputs = {"trans": trans, "rotat": rotat, "scal_dir": scal_dir, "scal": scal}
    # pad to BPAD with ones (zeros would make |v| = 0 -> inf/NaN chains)
    padded = {}
    for k, v in inputs.items():
        a = np.ones((BPAD, 3), dtype=np.float32)
        a[:B] = v
        padded[k] = a
    args = [padded[n] for n in in_names]
    zeros = [np.zeros((N_CORES * z.shape[0], *z.shape[1:]), z.dtype) for z in zero_outs]
    outs = jf(*args, *zeros)
    full = np.asarray(outs[0])  # [BPAD, 12]
    return full[:B].reshape(B, 3, 4).astype(np.float32, copy=False)


if __name__ == "__main__":
    rng = np.random.default_rng(0)
    ins = {
        "trans": rng.normal(size=(B, 3)).astype(np.float32),
        "rotat": rng.normal(size=(B, 3)).astype(np.float32),
        "scal_dir": rng.normal(size=(B, 3)).astype(np.float32),
        "scal": rng.normal(size=(B, 3)).astype(np.float32),
    }
    out = kernel(**ins)
    print(out.shape, out.dtype)
